# revision 71
# baseline (speedup 1.0000x reference)
"""Trainium2 Bass kernel for nn_MultiHeadAttention_66984309948505.

Full causal MHA: x[4,2048,1024], 16 heads of 64, out-proj + bias.

Sharding (8 cores): 4-way data-parallel over batch x 2-way tensor-parallel
over heads. Core (b, g) computes heads [8g, 8g+8) for batch b, including the
partial output projection Y_partial = O_g @ Wo[:, 512g:512(g+1)].T.
Host-side unshard: Y[b] = (Y_partial[b,g=0] + Y_partial[b,g=1]).T + bo.

Device layouts are all "transposed" (feature-major) so no on-chip transposes
are needed anywhere:
  xt  [c, p, dt, s] p-major tiling of x[b].T    (host pre-transpose)
  QT/KT fp8e4m3, DoubleRow-packed [128p = 4 heads x 32 lanes, grp, sub, T]:
                 head-dim element e of head h lives at partition
                 (h%4)*32 + e//2, group h//4, subrow e%2.  The host permutes
                 wq/wk s-columns so the projection matmuls PRODUCE this
                 layout directly (no repack).  Scores then run as fp8
                 DoubleRow matmuls (0.5 PE cycles/row — 2x bf16 rate).
  scores S^T[k, q] per head-pair; softmax denominator comes free from a
                 ones-column appended to V in the AV matmul (PSUM row 64)
  O^T [S, T], Y^T [D, T] in bf16 -> host transposes back.

Everything else is bf16 operands with fp32 PSUM accumulation (rel err
~9.8e-3, dominated by the fp8 Q/K quantization; gate is 2e-2).

Scheduling: attention is Activation(exp)-paced, so QKV-projection and
output-projection work is chopped into ~426ns "filler pieces" woven between
attention kt-steps by a rate+deadline Weaver; chunks 2+3 run hp-interleaved
as one merged phase so chunk 3's exp load shares chunk 2's filler pool.
Engine map: PE matmuls; ACT exp + some copies; DVE normalize chain + most
PSUM->SBUF copies; GpSimd/Pool causal-mask muls + memsets (Pool cannot
access PSUM on real HW).  DMAs are batched (each holds the single HWDGE
dispatcher ~625ns); y stores stage through SBUF and go out per chunk.
"""

import numpy as np

import concourse.bacc as bacc
import concourse.bass as bass
import concourse.mybir as mybir
import concourse.tile as tile
from concourse.bass_utils import run_bass_kernel_spmd

# Problem constants (hardcoded per contract)
B, T, D = 4, 2048, 1024
H, HS = 16, 64
NCORES = 8
HG = 2                 # head-group TP degree
H_LOC = H // HG        # 8 heads per core
S = H_LOC * HS         # 512 local head dims
P = 128
TCH = 512              # t/q chunk width
NCHUNK = T // TCH      # 4
ND = D // P            # 8 d-tiles
NSP = S // P           # 4 head stacks
NTT = TCH // P         # 4 k-subtiles per chunk
SCALE = 1.0 / np.sqrt(HS)

F32 = mybir.dt.float32
BF16 = mybir.dt.bfloat16
F8 = mybir.dt.float8e4
EXP = mybir.ActivationFunctionType.Exp
CPY = mybir.ActivationFunctionType.Copy
DROW = mybir.MatmulPerfMode.DoubleRow
NG = H_LOC // 4        # 2 four-head groups per core


class _Weaver:
    """Rate- and deadline-paced filler emission.  Each quantum is
    (closure, cost_us, deadline_call|None).  At pull() number k, emits (in
    list order) everything whose deadline is within `lead` calls, then keeps
    emitting while the cumulative cost is below rate_us * k."""

    def __init__(self, quanta, rate_us, lead=3):
        self.q = list(quanta)
        self.rate = rate_us
        self.lead = lead
        self.emitted = 0
        self.cost = 0.0
        self.calls = 0

    def _emit_next(self):
        fn, cost, _ = self.q[self.emitted]
        fn()
        self.cost += cost
        self.emitted += 1

    def __call__(self):
        self.calls += 1
        while self.emitted < len(self.q):
            _, cost, dl = self.q[self.emitted]
            due = dl is not None and dl <= self.calls + self.lead
            if due or self.cost < self.rate * self.calls:
                self._emit_next()
            else:
                break

    def drain(self):
        while self.emitted < len(self.q):
            self._emit_next()


def build_program(reps: int = 1, mmdt=BF16):
    nc = bacc.Bacc("TRN2", target_bir_lowering=False, debug=False)

    xt = nc.dram_tensor("xt", [NCHUNK, P, ND, TCH], mmdt, kind="ExternalInput")
    wq = nc.dram_tensor("wq", [P, ND, S], mmdt, kind="ExternalInput")
    wk = nc.dram_tensor("wk", [P, ND, S], mmdt, kind="ExternalInput")
    wv = nc.dram_tensor("wv", [P, ND, S], mmdt, kind="ExternalInput")
    wot = nc.dram_tensor("wot", [P, NSP, D], mmdt, kind="ExternalInput")
    tri = nc.dram_tensor("tri", [P, P], mmdt, kind="ExternalInput")
    yt = nc.dram_tensor("yt", [ND, P, T], mmdt, kind="ExternalOutput")

    with tile.TileContext(nc) as tc:
        with (
            nc.allow_low_precision(reason="bf16 matmul operands, fp32 accum"),
            tc.tile_pool(name="const", bufs=1) as constp,
            tc.tile_pool(name="kv", bufs=1) as kvp,
            tc.tile_pool(name="qt", bufs=2) as qtp,
            tc.tile_pool(name="osb", bufs=4) as osbp,
            tc.tile_pool(name="xp", bufs=3) as xp,
            tc.tile_pool(name="ptp", bufs=3) as ptp,
            tc.tile_pool(name="bcp", bufs=2) as bcp,
            tc.tile_pool(name="stg", bufs=2) as stp,
            tc.tile_pool(name="psS", bufs=2, space="PSUM") as psS,
            tc.tile_pool(name="psO", bufs=3, space="PSUM") as psO,
            tc.tile_pool(name="psW", bufs=1, space="PSUM") as psW,
        ):
            # Resident weights / constants
            tri_sb = constp.tile([P, P], mmdt, name="tri_sb")
            rcp_st = constp.tile([P, TCH], F32, name="rcp_st")
            nc.gpsimd.memset(rcp_st[:], 1.0)
            wq_sb = constp.tile([P, ND, S], mmdt, name="wq_sb")
            wk_sb = constp.tile([P, ND, S], mmdt, name="wk_sb")
            wv_sb = constp.tile([P, ND, S], mmdt, name="wv_sb")
            wot_sb = constp.tile([P, NSP, D], mmdt, name="wot_sb")

            xt_tiles = {}

            def emit_x_dma(c):
                if c not in xt_tiles:
                    xt_tiles[c] = xp.tile([P, ND, TCH], mmdt, tag="x",
                                          name=f"x{c}")
                    nc.sync.dma_start(out=xt_tiles[c][:], in_=xt[c])

            if reps == 1:
                # Startup: interleave x0 / wq in small pieces so the
                # dt-outer chunk-0 Q loop starts ~3us in; the g0 half of wk
                # comes early (K0 g0-stacks run right after Q0 g0), then wv
                # (V0 weaves into chunk 0 with tight deadlines), rest after.
                xt_tiles[0] = xp.tile([P, ND, TCH], mmdt, tag="x", name="x0")
                for sl in (slice(0, 1), slice(1, 4)):
                    nc.sync.dma_start(out=xt_tiles[0][:, sl, :],
                                      in_=xt[0, :, sl, :])
                    nc.sync.dma_start(out=wq_sb[:, sl, :], in_=wq[:, sl, :])
                nc.sync.dma_start(out=wk_sb[:, :, 0:2 * P],
                                  in_=wk[:, :, 0:2 * P])
                sl = slice(4, 8)
                nc.sync.dma_start(out=xt_tiles[0][:, sl, :],
                                  in_=xt[0, :, sl, :])
                nc.sync.dma_start(out=wq_sb[:, sl, :], in_=wq[:, sl, :])
                nc.sync.dma_start(out=wv_sb[:], in_=wv[:])
                nc.sync.dma_start(out=wk_sb[:, :, 2 * P:],
                                  in_=wk[:, :, 2 * P:])
            else:
                nc.sync.dma_start(out=wq_sb[:], in_=wq[:])
                nc.sync.dma_start(out=wk_sb[:], in_=wk[:])
                nc.sync.dma_start(out=wv_sb[:], in_=wv[:])
            nc.sync.dma_start(out=tri_sb[:], in_=tri[:])
            nc.sync.dma_start(out=wot_sb[:], in_=wot[:])

            # Resident K^T and V (per-chunk tiles for clean dep tracking);
            # V column 64 = 1.0 -> AV PSUM row 64 is the softmax denominator.
            # Q^T/K^T live in fp8e4m3 packed for DoubleRow score matmuls:
            # [128p = 4 heads x 32 lanes, group, subrow, T], head-dim element
            # e of head h at (partition (h%4)*32 + e//2, group h//4, sub e%2).
            kt_sb = [kvp.tile([P, NG, 2, TCH], F8, name=f"kt{c}")
                     for c in range(NCHUNK)]
            v_sb = [kvp.tile([P, NTT, H_LOC, HS + 1], mmdt, name=f"v{c}")
                    for c in range(NCHUNK)]
            for c in range(NCHUNK):
                nc.gpsimd.memset(v_sb[c][:, :, :, HS:HS + 1], 1.0)

            qt_tiles = {}
            o_tiles = {}
            y_stage = {}

            def ensure_qt(c):
                if c not in qt_tiles:
                    qt_tiles[c] = qtp.tile([P, NG, 2, TCH], F8, tag="qt",
                                           name=f"qt{c}")

            def cp_act(dst, src):
                nc.scalar.activation(dst, src, CPY)

            pending_ps = {}

            def emit_qk_stack(c, which, st, cp=cp_act, pool=None, part=None,
                              nparts=4):
                # stack st = (group g = st//2, subrow r = st%2); the host
                # permutes wq/wk s-columns to match, so the matmul output
                # rows land directly in fp8-DoubleRow packing order.
                # part=i of nparts emits a dt-slice of the shared PSUM
                # accumulation so filler weaves at fine (~426ns) grain.
                ensure_qt(c)
                w_sb, dst = ((wq_sb, qt_tiles[c]), (wk_sb, kt_sb[c]))[which]
                pool = pool or psW
                key = ("qk", c, which, st)
                if part in (None, 0):
                    ps = (pool.tile([P, TCH], F32, tag="w", name="ps_a")
                          if pool is psW else
                          pool.tile([P, TCH], F32, tag="o", name="o_ps"))
                else:
                    ps = pending_ps.pop(key)
                step = ND // nparts if part is not None else ND
                d0 = 0 if part is None else part * step
                for dt in range(d0, d0 + step):
                    nc.tensor.matmul(
                        ps[:], w_sb[:, dt, st * P:(st + 1) * P],
                        xt_tiles[c][:, dt, :],
                        start=(dt == 0), stop=(dt == ND - 1))
                if part is not None and part < nparts - 1:
                    pending_ps[key] = ps
                else:
                    cp(dst[:, st // 2, st % 2, :], ps[:])

            def emit_qk0_pair(which, pair):
                # dt-outer over two stacks: round dt only needs the dt-piece
                # of (x0, w) -> PE starts as soon as the first DMAs land.
                # Borrows a psO slot for the second live accumulator.
                ensure_qt(0)
                w_sb, dst = ((wq_sb, qt_tiles[0]), (wk_sb, kt_sb[0]))[which]
                pss = [psW.tile([P, TCH], F32, tag="w", name="ps_a"),
                       psO.tile([P, TCH], F32, tag="o", name="o_ps")]
                for dt in range(ND):
                    for i in range(2):
                        st = 2 * pair + i
                        nc.tensor.matmul(
                            pss[i][:], w_sb[:, dt, st * P:(st + 1) * P],
                            xt_tiles[0][:, dt, :],
                            start=(dt == 0), stop=(dt == ND - 1))
                for i in range(2):
                    st = 2 * pair + i
                    (cp_act if i else nc.vector.tensor_copy)(
                        dst[:, st // 2, st % 2, :], pss[i][:])

            def emit_v_tt(c, tt, pool=None, part=None, nparts=4):
                pool = pool or psW
                key = ("v", c, tt)
                if part in (None, 0):
                    ps = (pool.tile([P, TCH], F32, tag="w", name="ps_v")
                          if pool is psW else
                          pool.tile([P, TCH], F32, tag="o", name="o_ps"))
                else:
                    ps = pending_ps.pop(key)
                step = ND // nparts if part is not None else ND
                d0 = 0 if part is None else part * step
                for dt in range(d0, d0 + step):
                    nc.tensor.matmul(
                        ps[:], xt_tiles[c][:, dt, tt * P:(tt + 1) * P],
                        wv_sb[:, dt, :],
                        start=(dt == 0), stop=(dt == ND - 1))
                if part is not None and part < nparts - 1:
                    pending_ps[key] = ps
                else:
                    nc.vector.tensor_copy(
                        v_sb[c][:, tt, :, 0:HS],
                        ps[:].rearrange("p (h e) -> p h e", h=H_LOC))

            def ensure_ystage(c):
                if c not in y_stage:
                    y_stage[c] = stp.tile([P, ND, TCH], mmdt, tag="y",
                                          name=f"yst{c}")

            def emit_proj_et(c, et, part=None):
                ensure_ystage(c)
                key = ("p", c, et)
                if part in (None, 0):
                    y_ps = psW.tile([P, TCH], F32, tag="w", name="y_ps")
                else:
                    y_ps = pending_ps.pop(key)
                sps = (range(NSP) if part is None else
                       range(0, 2) if part == 0 else range(2, NSP))
                for sp in sps:
                    nc.tensor.matmul(
                        y_ps[:], wot_sb[:, sp, et * P:(et + 1) * P],
                        o_tiles[c][:, sp, :],
                        start=(sp == 0), stop=(sp == NSP - 1))
                if part == 0:
                    pending_ps[key] = y_ps
                    return
                nc.vector.tensor_copy(y_stage[c][:, et, :], y_ps[:])
                if et == ND - 1:
                    nc.sync.dma_start(
                        out=yt[:, :, c * TCH:(c + 1) * TCH]
                        .rearrange("e p t -> p e t"),
                        in_=y_stage[c][:])

            def emit_proj_tail(c):
                # et-pairs with deferred sp3: PE accumulates stacks 0-2 of
                # both ets while the last hp's normalize (which produces
                # stack 3) drains on DVE; copies alternate DVE/ACT; y goes
                # out in two half-chunk DMAs.
                ensure_ystage(c)
                for pair in range(ND // 2):
                    e0, e1 = 2 * pair, 2 * pair + 1
                    pss = {e0: psW.tile([P, TCH], F32, tag="w", name="y_ps"),
                           e1: psO.tile([P, TCH], F32, tag="o", name="o_ps")}
                    for et in (e0, e1):
                        for sp in range(NSP - 1):
                            nc.tensor.matmul(
                                pss[et][:],
                                wot_sb[:, sp, et * P:(et + 1) * P],
                                o_tiles[c][:, sp, :],
                                start=(sp == 0), stop=False)
                    for et in (e0, e1):
                        nc.tensor.matmul(
                            pss[et][:],
                            wot_sb[:, NSP - 1, et * P:(et + 1) * P],
                            o_tiles[c][:, NSP - 1, :],
                            start=False, stop=True)
                        cp = nc.vector.tensor_copy if et % 2 == 0 else cp_act
                        cp(y_stage[c][:, et, :], pss[et][:])
                    if pair == 2:
                        nc.sync.dma_start(
                            out=yt[0:6, :, c * TCH:(c + 1) * TCH]
                            .rearrange("e p t -> p e t"),
                            in_=y_stage[c][:, 0:6, :])
                    elif pair == 3:
                        nc.sync.dma_start(
                            out=yt[6:8, :, c * TCH:(c + 1) * TCH]
                            .rearrange("e p t -> p e t"),
                            in_=y_stage[c][:, 6:8, :])

            def emit_hp(c, hp, o_sb_c, pull):
                qt_c = qt_tiles[c]
                nkt = 4 * c + 4
                o_ps = [psO.tile([P, TCH], F32, tag="o", name="o_ps")
                        for _ in range(2)]

                def emit_av(kt, pt, q0):
                    cc, tt = kt // 4, kt % 4
                    for j in range(2):
                        h = 2 * hp + j
                        nc.tensor.matmul(
                            o_ps[j][0:HS + 1, q0:],
                            v_sb[cc][:, tt, h, :],
                            pt[:, j, q0:],
                            start=(kt == 0), stop=(kt == nkt - 1))

                # software-pipelined: scores(kt+1)/(kt+2) enter the PE queue
                # BEFORE av(kt) so the exp(kt) + mask(kt) latency hides
                # behind them; one filler quantum is woven in per kt step.
                pends = []
                for kt in range(nkt):
                    cc, tt = kt // 4, kt % 4
                    q0 = max(0, P * kt - TCH * c)
                    s_ps = psS.tile([P, 2, TCH], F32, tag="s", name="s_ps")
                    for j in range(2):
                        h = 2 * hp + j
                        hb = (h % 4) * 32
                        g = h // 4
                        nc.tensor.matmul(
                            s_ps[:, j, q0:],
                            kt_sb[cc][hb:hb + 32, g, :, tt * P:(tt + 1) * P],
                            qt_c[hb:hb + 32, g, :, q0:],
                            start=True, stop=True, perf_mode=DROW,
                            tile_position=(hb, 0))
                    pt = ptp.tile([P, 2, TCH], mmdt, tag="pt", name="pt")
                    nc.scalar.activation(
                        pt[:, :, q0:], s_ps[:, :, q0:], EXP, scale=float(SCALE))
                    if kt >= 4 * c:  # diagonal block: causal tri mask
                        # on GpSimd/Pool: SBUF-only op, keeps the in-order
                        # DVE queue free of latency-critical work
                        for j in range(2):
                            nc.gpsimd.tensor_mul(
                                pt[:, j, q0:q0 + P], pt[:, j, q0:q0 + P],
                                tri_sb[:])
                    if len(pends) >= 2:
                        emit_av(*pends.pop(0))
                    pends.append((kt, pt, q0))
                    pull()
                for pend in pends:
                    emit_av(*pend)
                # normalize: rows 0:64 of o_ps / row 64 (ones-column rowsum).
                # All-DVE chain: recip -> 2x stream_shuffle lane-0 broadcast
                # -> mul.  rcp/bc in bf16 so the shuffles hit the fast
                # 16-bit DVE modes.
                for j in range(2):
                    nc.vector.reciprocal(rcp_st[64:65, :], o_ps[j][64:65, :])
                    bc_sb = bcp.tile([P, TCH], F32, tag="bc", name="bc_sb")
                    nc.vector.stream_shuffle(
                        bc_sb[0:32, :], rcp_st[64:96, :], [0] * 32)
                    nc.vector.stream_shuffle(
                        bc_sb[32:64, :], rcp_st[64:96, :], [0] * 32)
                    nc.vector.tensor_mul(
                        o_sb_c[j * 64:(j + 1) * 64, hp, :],
                        o_ps[j][0:64, :], bc_sb[0:64, :])

            def emit_body():
                # Startup: only what chunk-0's first two head-pairs (group
                # g0) need — Q0/K0 g0 stacks + V0; the g1 stacks weave into
                # chunk 0 (deadline: hp2).  First Q pair dt-outer for the
                # earliest possible PE start; copies alternate DVE/ACT.
                emit_x_dma(0)
                emit_qk0_pair(0, 0)
                cps = [nc.vector.tensor_copy, cp_act]
                pools = [psO, psO, psW]
                for i, (which, st) in enumerate([(1, 0), (1, 1)]):
                    emit_qk_stack(0, which, st, cp=cps[i % 2],
                                  pool=pools[i % 3])
                for tt in range(NTT):
                    emit_v_tt(0, tt, pool=pools[tt % 3])

                # Filler schedule.  Attention is ACT(exp)-paced after the
                # fp8 scores change (~0.5us/kt PE deficit), so fillers are
                # paced at that rate with explicit deadlines:
                #  - K(c)/V(c) weave inside chunk c (needed by its diag kts)
                #  - Q(c+1) weaves in chunk c (needed at chunk c+1 start)
                #  - proj(c) defers into the merged phase
                # Chunks 2+3 run as ONE interleaved phase (hp-alternating)
                # so chunk 3's huge exp load shares chunk 2's filler pool.
                dve_cp = nc.vector.tensor_copy
                PC = 0.426  # fine filler piece cost (us): 2 matmuls

                def qk(c, which, st, dl, cp=None):
                    # four quarter-stack pieces sharing one PSUM accumulation
                    cp = cp or dve_cp
                    return [(lambda p=p: emit_qk_stack(c, which, st, cp,
                                                       part=p), PC, dl)
                            for p in range(4)]

                def vq(c, tt, dl):
                    return [(lambda p=p: emit_v_tt(c, tt, part=p), PC, dl)
                            for p in range(4)]

                def pj(c, et):
                    return [(lambda p=p: emit_proj_et(c, et, part=p), PC,
                             None) for p in range(2)]

                def flat(groups):
                    return [q for grp in groups for q in grp]

                RATE = 0.5
                # --- chunk 0 (16 calls) --- DVE is loaded with normalize
                # here while ACT has slack: QK copies go to ACT.  V0 tt1-3
                # weave in with tight deadlines (diag kts 1-3 of hp0).
                o_tiles[0] = osbp.tile([P, NSP, TCH], mmdt, tag="o", name="o0")
                emit_x_dma(1)
                w = _Weaver(flat([qk(0, 0, 2, 9, cp_act), qk(0, 1, 2, 9, cp_act),
                                  qk(0, 0, 3, 9, cp_act), qk(0, 1, 3, 9, cp_act),
                                  qk(1, 0, 0, 14, cp_act), qk(1, 0, 1, 14, cp_act),
                                  qk(1, 0, 2, None, cp_act),
                                  qk(1, 0, 3, None, cp_act)]),
                            RATE)
                for hp in range(4):
                    emit_hp(0, hp, o_tiles[0], w)
                w.drain()
                # --- chunk 1 (32 calls) ---
                o_tiles[1] = osbp.tile([P, NSP, TCH], mmdt, tag="o", name="o1")
                emit_x_dma(2)
                emit_x_dma(3)
                w = _Weaver(flat([qk(1, 1, 0, 5, cp_act), qk(1, 1, 1, 5, cp_act),
                                  vq(1, 0, 5), vq(1, 1, 6), vq(1, 2, 7),
                                  vq(1, 3, 8),
                                  qk(1, 1, 2, 21, cp_act),
                                  qk(1, 1, 3, 21, cp_act),
                                  qk(2, 0, 0, 30, cp_act),
                                  qk(2, 0, 1, 30, cp_act),
                                  qk(2, 0, 2, None, cp_act),
                                  qk(2, 0, 3, None, cp_act)]),
                            RATE)
                for hp in range(4):
                    emit_hp(1, hp, o_tiles[1], w)
                w.drain()
                # --- merged chunks 2+3 (112 calls, hp-alternating) ---
                o_tiles[2] = osbp.tile([P, NSP, TCH], mmdt, tag="o", name="o2")
                o_tiles[3] = osbp.tile([P, NSP, TCH], mmdt, tag="o", name="o3")
                quanta = flat([qk(2, 1, 0, 9), qk(2, 1, 1, 9),
                               vq(2, 0, 9), vq(2, 1, 10), vq(2, 2, 11),
                               vq(2, 3, 12),
                               qk(3, 0, 0, 13), qk(3, 0, 1, 13),
                               qk(3, 1, 0, 25), qk(3, 1, 1, 25),
                               vq(3, 0, 25), vq(3, 1, 26), vq(3, 2, 27),
                               vq(3, 3, 28),
                               qk(2, 1, 2, 37), qk(2, 1, 3, 37),
                               qk(3, 0, 2, 41), qk(3, 0, 3, 41),
                               qk(3, 1, 2, 53), qk(3, 1, 3, 53)]
                              + [pj(0, et) for et in range(ND)]
                              + [pj(1, et) for et in range(ND)]
                              + [pj(2, et) for et in range(ND)])
                w = _Weaver(quanta, 0.58)
                for hp in range(4):
                    emit_hp(2, hp, o_tiles[2], w)
                    emit_hp(3, hp, o_tiles[3], w)
                w.drain()

                # tail: last chunk's projection. sp3 (the stack normalized
                # last) is deferred per et-pair so PE keeps busy on the
                # other stacks while the final normalize chain drains.
                emit_proj_tail(NCHUNK - 1)

            import contextlib
            loop_ctx = (tc.For_i(0, reps, 1) if reps > 1
                        else contextlib.nullcontext())
            with loop_ctx:
                emit_body()

    nc.compile()
    return nc


_CACHE = {}


def _get_program(reps: int = 1, mmdt=BF16):
    key = ("nc", reps, str(mmdt))
    if key not in _CACHE:
        _CACHE[key] = build_program(reps, mmdt)
    return _CACHE[key]


def make_in_maps(x, Wq, Wk, Wv, Wo, npdt=None):
    if npdt is None:
        import ml_dtypes
        npdt = ml_dtypes.bfloat16
    x = np.asarray(x, dtype=np.float32)
    Wq = np.asarray(Wq, dtype=np.float32)
    Wk = np.asarray(Wk, dtype=np.float32)
    Wv = np.asarray(Wv, dtype=np.float32)
    Wo = np.asarray(Wo, dtype=np.float32)
    tri = np.triu(np.ones((P, P), dtype=np.float32))

    def wmat(W, g):
        # [H_LOC, D, HS] -> [D, S] (s = h_local*HS + e) -> p-major [P, ND, S]
        m = W[g * H_LOC:(g + 1) * H_LOC].transpose(1, 0, 2).reshape(D, S)
        return np.ascontiguousarray(m.reshape(ND, P, S).transpose(1, 0, 2))

    # fp8-DoubleRow packing permutation for wq/wk: stack st = (grp, subrow),
    # column c -> head 4*grp + c//32, head-dim element e = 2*(c%32) + subrow
    perm = np.empty(S, dtype=np.int64)
    for st in range(4):
        grp, r = st // 2, st % 2
        for cc in range(P):
            perm[st * P + cc] = (4 * grp + cc // 32) * HS + 2 * (cc % 32) + r

    def wmat_qk(W, g):
        m = W[g * H_LOC:(g + 1) * H_LOC].transpose(1, 0, 2).reshape(D, S)
        m = m[:, perm]
        return np.ascontiguousarray(m.reshape(ND, P, S).transpose(1, 0, 2))

    in_maps = []
    for core in range(NCORES):
        b, g = core // HG, core % HG
        xT = x[b].T  # [D, T]
        xt_t = np.ascontiguousarray(
            xT.reshape(ND, P, NCHUNK, TCH).transpose(2, 1, 0, 3))
        woT = Wo[:, g * S:(g + 1) * S].T  # [S, D]
        wot_t = np.ascontiguousarray(woT.reshape(NSP, P, D).transpose(1, 0, 2))
        in_maps.append({
            "xt": xt_t.astype(npdt),
            "wq": wmat_qk(Wq, g).astype(npdt),
            "wk": wmat_qk(Wk, g).astype(npdt),
            "wv": wmat(Wv, g).astype(npdt),
            "wot": wot_t.astype(npdt),
            "tri": tri.astype(npdt),
        })
    return in_maps


def kernel_ex(x, Wq, Wk, Wv, Wo, bo, **run_kwargs):
    """Run and return (output, BassKernelResults)."""
    nc = _get_program()
    in_maps = make_in_maps(x, Wq, Wk, Wv, Wo)
    res = run_bass_kernel_spmd(nc, in_maps, core_ids=list(range(NCORES)),
                               **run_kwargs)
    bo = np.asarray(bo, dtype=np.float32)
    y = np.empty((B, T, D), dtype=np.float32)
    for b in range(B):
        y0 = res.results[HG * b]["yt"].astype(np.float32).reshape(D, T)
        y1 = res.results[HG * b + 1]["yt"].astype(np.float32).reshape(D, T)
        y[b] = (y0 + y1).T + bo
    return y, res


def kernel(x, Wq, Wk, Wv, Wo, bo):
    y, _ = kernel_ex(x, Wq, Wk, Wv, Wo, bo)
    return y


# revision 79
# speedup vs baseline: 1.0099x; 1.0099x over previous
"""Trainium2 Bass kernel for nn_MultiHeadAttention_66984309948505.

Full causal MHA: x[4,2048,1024], 16 heads of 64, out-proj + bias.

Sharding (8 cores): 4-way data-parallel over batch x 2-way tensor-parallel
over heads. Core (b, g) computes heads [8g, 8g+8) for batch b, including the
partial output projection Y_partial = O_g @ Wo[:, 512g:512(g+1)].T.
Host-side unshard: Y[b] = (Y_partial[b,g=0] + Y_partial[b,g=1]).T + bo.

Device layouts are all "transposed" (feature-major) so no on-chip transposes
are needed anywhere:
  xt  [c, p, dt, s] p-major tiling of x[b].T    (host pre-transpose)
  QT/KT fp8e4m3, DoubleRow-packed [128p = 4 heads x 32 lanes, grp, sub, T]:
                 head-dim element e of head h lives at partition
                 (h%4)*32 + e//2, group h//4, subrow e%2.  The host permutes
                 wq/wk s-columns so the projection matmuls PRODUCE this
                 layout directly (no repack).  Scores then run as fp8
                 DoubleRow matmuls (0.5 PE cycles/row — 2x bf16 rate).
  scores S^T[k, q] per head-pair; softmax denominator comes free from a
                 ones-column appended to V in the AV matmul (PSUM row 64)
  O^T [S, T], Y^T [D, T] in bf16 -> host transposes back.

Everything else is bf16 operands with fp32 PSUM accumulation (rel err
~9.8e-3, dominated by the fp8 Q/K quantization; gate is 2e-2).

Scheduling: attention is Activation(exp)-paced, so QKV-projection and
output-projection work is chopped into ~426ns "filler pieces" woven between
attention kt-steps by a rate+deadline Weaver; chunks 2+3 run hp-interleaved
as one merged phase so chunk 3's exp load shares chunk 2's filler pool.
Engine map: PE matmuls; ACT exp + some copies; DVE normalize chain + most
PSUM->SBUF copies; GpSimd/Pool causal-mask muls + memsets (Pool cannot
access PSUM on real HW).  DMAs are batched (each holds the single HWDGE
dispatcher ~625ns); y stores stage through SBUF and go out per chunk.
"""

import numpy as np

import concourse.bacc as bacc
import concourse.bass as bass
import concourse.mybir as mybir
import concourse.tile as tile
from concourse.bass_utils import run_bass_kernel_spmd

# Problem constants (hardcoded per contract)
B, T, D = 4, 2048, 1024
H, HS = 16, 64
NCORES = 8
HG = 2                 # head-group TP degree
H_LOC = H // HG        # 8 heads per core
S = H_LOC * HS         # 512 local head dims
P = 128
TCH = 512              # t/q chunk width
NCHUNK = T // TCH      # 4
ND = D // P            # 8 d-tiles
NSP = S // P           # 4 head stacks
NTT = TCH // P         # 4 k-subtiles per chunk
SCALE = 1.0 / np.sqrt(HS)

F32 = mybir.dt.float32
BF16 = mybir.dt.bfloat16
F8 = mybir.dt.float8e4
EXP = mybir.ActivationFunctionType.Exp
CPY = mybir.ActivationFunctionType.Copy
DROW = mybir.MatmulPerfMode.DoubleRow
NG = H_LOC // 4        # 2 four-head groups per core


class _Weaver:
    """Rate- and deadline-paced filler emission.  Each quantum is
    (closure, cost_us, deadline_call|None).  At pull() number k, emits (in
    list order) everything whose deadline is within `lead` calls, then keeps
    emitting while the cumulative cost is below rate_us * k."""

    def __init__(self, quanta, rate_us, lead=3):
        self.q = list(quanta)
        self.rate = rate_us
        self.lead = lead
        self.emitted = 0
        self.cost = 0.0
        self.calls = 0

    def _emit_next(self):
        fn, cost, _ = self.q[self.emitted]
        fn()
        self.cost += cost
        self.emitted += 1

    def __call__(self):
        self.calls += 1
        while self.emitted < len(self.q):
            _, cost, dl = self.q[self.emitted]
            due = dl is not None and dl <= self.calls + self.lead
            if due or self.cost < self.rate * self.calls:
                self._emit_next()
            else:
                break

    def drain(self):
        while self.emitted < len(self.q):
            self._emit_next()


def build_program(reps: int = 1, mmdt=BF16):
    nc = bacc.Bacc("TRN2", target_bir_lowering=False, debug=False)

    xt = nc.dram_tensor("xt", [NCHUNK, P, ND, TCH], mmdt, kind="ExternalInput")
    wq = nc.dram_tensor("wq", [P, ND, S], mmdt, kind="ExternalInput")
    wk = nc.dram_tensor("wk", [P, ND, S], mmdt, kind="ExternalInput")
    wv = nc.dram_tensor("wv", [P, ND, S], mmdt, kind="ExternalInput")
    wot = nc.dram_tensor("wot", [P, NSP, D], mmdt, kind="ExternalInput")
    tri = nc.dram_tensor("tri", [P, P], mmdt, kind="ExternalInput")
    yt = nc.dram_tensor("yt", [ND, P, T], mmdt, kind="ExternalOutput")

    with tile.TileContext(nc) as tc:
        with (
            nc.allow_low_precision(reason="bf16 matmul operands, fp32 accum"),
            tc.tile_pool(name="const", bufs=1) as constp,
            tc.tile_pool(name="kv", bufs=1) as kvp,
            tc.tile_pool(name="qt", bufs=2) as qtp,
            tc.tile_pool(name="osb", bufs=4) as osbp,
            tc.tile_pool(name="xp", bufs=3) as xp,
            tc.tile_pool(name="ptp", bufs=3) as ptp,
            tc.tile_pool(name="bcp", bufs=2) as bcp,
            tc.tile_pool(name="stg", bufs=2) as stp,
            tc.tile_pool(name="psS", bufs=2, space="PSUM") as psS,
            tc.tile_pool(name="psO", bufs=3, space="PSUM") as psO,
            tc.tile_pool(name="psW", bufs=1, space="PSUM") as psW,
        ):
            # Resident weights / constants
            tri_sb = constp.tile([P, P], mmdt, name="tri_sb")
            rcp_st = constp.tile([P, TCH], F32, name="rcp_st")
            nc.gpsimd.memset(rcp_st[:], 1.0)
            # PE p-state warmup: junk matmuls on memset data burn the
            # 0.65/1.2GHz ramp during the initial DMA wait, so the first
            # real matmuls run at full clock
            warm_sb = constp.tile([P, 256], mmdt, name="warm_sb")
            nc.gpsimd.memset(warm_sb[:], 0.0)
            warm_ps = psW.tile([P, TCH], F32, tag="w", name="warm_ps")
            for i in range(8):
                nc.tensor.matmul(warm_ps[:, 0:256], warm_sb[:, 0:P], warm_sb[:],
                                 start=(i == 0), stop=(i == 8 - 1))
            wq_sb = constp.tile([P, ND, S], mmdt, name="wq_sb")
            wk_sb = constp.tile([P, ND, S], mmdt, name="wk_sb")
            wv_sb = constp.tile([P, ND, S], mmdt, name="wv_sb")
            wot_sb = constp.tile([P, NSP, D], mmdt, name="wot_sb")

            xt_tiles = {}

            def emit_x_dma(c):
                if c not in xt_tiles:
                    xt_tiles[c] = xp.tile([P, ND, TCH], mmdt, tag="x",
                                          name=f"x{c}")
                    nc.sync.dma_start(out=xt_tiles[c][:], in_=xt[c])

            if reps == 1:
                # Startup: interleave x0 / wq in small pieces so the
                # dt-outer chunk-0 Q loop starts ~3us in; the g0 half of wk
                # comes early (K0 g0-stacks run right after Q0 g0), then wv
                # (V0 weaves into chunk 0 with tight deadlines), rest after.
                xt_tiles[0] = xp.tile([P, ND, TCH], mmdt, tag="x", name="x0")
                for sl in (slice(0, 1), slice(1, 4)):
                    nc.sync.dma_start(out=xt_tiles[0][:, sl, :],
                                      in_=xt[0, :, sl, :])
                    nc.sync.dma_start(out=wq_sb[:, sl, :], in_=wq[:, sl, :])
                nc.sync.dma_start(out=wk_sb[:, :, 0:2 * P],
                                  in_=wk[:, :, 0:2 * P])
                sl = slice(4, 8)
                nc.sync.dma_start(out=xt_tiles[0][:, sl, :],
                                  in_=xt[0, :, sl, :])
                nc.sync.dma_start(out=wq_sb[:, sl, :], in_=wq[:, sl, :])
                nc.sync.dma_start(out=wv_sb[:], in_=wv[:])
                nc.sync.dma_start(out=wk_sb[:, :, 2 * P:],
                                  in_=wk[:, :, 2 * P:])
            else:
                nc.sync.dma_start(out=wq_sb[:], in_=wq[:])
                nc.sync.dma_start(out=wk_sb[:], in_=wk[:])
                nc.sync.dma_start(out=wv_sb[:], in_=wv[:])
            nc.sync.dma_start(out=tri_sb[:], in_=tri[:])
            nc.sync.dma_start(out=wot_sb[:], in_=wot[:])

            # Resident K^T and V (per-chunk tiles for clean dep tracking);
            # V column 64 = 1.0 -> AV PSUM row 64 is the softmax denominator.
            # Q^T/K^T live in fp8e4m3 packed for DoubleRow score matmuls:
            # [128p = 4 heads x 32 lanes, group, subrow, T], head-dim element
            # e of head h at (partition (h%4)*32 + e//2, group h//4, sub e%2).
            kt_sb = [kvp.tile([P, NG, 2, TCH], F8, name=f"kt{c}")
                     for c in range(NCHUNK)]
            v_sb = [kvp.tile([P, NTT, H_LOC, HS + 1], mmdt, name=f"v{c}")
                    for c in range(NCHUNK)]
            for c in range(NCHUNK):
                nc.gpsimd.memset(v_sb[c][:, :, :, HS:HS + 1], 1.0)

            qt_tiles = {}
            o_tiles = {}
            y_stage = {}

            def ensure_qt(c):
                if c not in qt_tiles:
                    qt_tiles[c] = qtp.tile([P, NG, 2, TCH], F8, tag="qt",
                                           name=f"qt{c}")

            def cp_act(dst, src):
                nc.scalar.activation(dst, src, CPY)

            pending_ps = {}

            def emit_qk_stack(c, which, st, cp=cp_act, pool=None, part=None,
                              nparts=4):
                # stack st = (group g = st//2, subrow r = st%2); the host
                # permutes wq/wk s-columns to match, so the matmul output
                # rows land directly in fp8-DoubleRow packing order.
                # part=i of nparts emits a dt-slice of the shared PSUM
                # accumulation so filler weaves at fine (~426ns) grain.
                ensure_qt(c)
                w_sb, dst = ((wq_sb, qt_tiles[c]), (wk_sb, kt_sb[c]))[which]
                pool = pool or psW
                key = ("qk", c, which, st)
                if part in (None, 0):
                    ps = (pool.tile([P, TCH], F32, tag="w", name="ps_a")
                          if pool is psW else
                          pool.tile([P, TCH], F32, tag="o", name="o_ps"))
                else:
                    ps = pending_ps.pop(key)
                step = ND // nparts if part is not None else ND
                d0 = 0 if part is None else part * step
                for dt in range(d0, d0 + step):
                    nc.tensor.matmul(
                        ps[:], w_sb[:, dt, st * P:(st + 1) * P],
                        xt_tiles[c][:, dt, :],
                        start=(dt == 0), stop=(dt == ND - 1))
                if part is not None and part < nparts - 1:
                    pending_ps[key] = ps
                else:
                    cp(dst[:, st // 2, st % 2, :], ps[:])

            def emit_qk0_pair(which, pair):
                # dt-outer over two stacks: round dt only needs the dt-piece
                # of (x0, w) -> PE starts as soon as the first DMAs land.
                # Borrows a psO slot for the second live accumulator.
                ensure_qt(0)
                w_sb, dst = ((wq_sb, qt_tiles[0]), (wk_sb, kt_sb[0]))[which]
                pss = [psW.tile([P, TCH], F32, tag="w", name="ps_a"),
                       psO.tile([P, TCH], F32, tag="o", name="o_ps")]
                for dt in range(ND):
                    for i in range(2):
                        st = 2 * pair + i
                        nc.tensor.matmul(
                            pss[i][:], w_sb[:, dt, st * P:(st + 1) * P],
                            xt_tiles[0][:, dt, :],
                            start=(dt == 0), stop=(dt == ND - 1))
                for i in range(2):
                    st = 2 * pair + i
                    (cp_act if i else nc.vector.tensor_copy)(
                        dst[:, st // 2, st % 2, :], pss[i][:])

            def emit_v_tt(c, tt, pool=None, part=None, nparts=4):
                pool = pool or psW
                key = ("v", c, tt)
                if part in (None, 0):
                    ps = (pool.tile([P, TCH], F32, tag="w", name="ps_v")
                          if pool is psW else
                          pool.tile([P, TCH], F32, tag="o", name="o_ps"))
                else:
                    ps = pending_ps.pop(key)
                step = ND // nparts if part is not None else ND
                d0 = 0 if part is None else part * step
                for dt in range(d0, d0 + step):
                    nc.tensor.matmul(
                        ps[:], xt_tiles[c][:, dt, tt * P:(tt + 1) * P],
                        wv_sb[:, dt, :],
                        start=(dt == 0), stop=(dt == ND - 1))
                if part is not None and part < nparts - 1:
                    pending_ps[key] = ps
                else:
                    nc.vector.tensor_copy(
                        v_sb[c][:, tt, :, 0:HS],
                        ps[:].rearrange("p (h e) -> p h e", h=H_LOC))

            def ensure_ystage(c):
                if c not in y_stage:
                    y_stage[c] = stp.tile([P, ND, TCH], mmdt, tag="y",
                                          name=f"yst{c}")

            def emit_proj_et(c, et, part=None):
                ensure_ystage(c)
                key = ("p", c, et)
                if part in (None, 0):
                    y_ps = psW.tile([P, TCH], F32, tag="w", name="y_ps")
                else:
                    y_ps = pending_ps.pop(key)
                sps = (range(NSP) if part is None else
                       range(0, 2) if part == 0 else range(2, NSP))
                for sp in sps:
                    nc.tensor.matmul(
                        y_ps[:], wot_sb[:, sp, et * P:(et + 1) * P],
                        o_tiles[c][:, sp, :],
                        start=(sp == 0), stop=(sp == NSP - 1))
                if part == 0:
                    pending_ps[key] = y_ps
                    return
                nc.vector.tensor_copy(y_stage[c][:, et, :], y_ps[:])
                if et == ND - 1:
                    nc.sync.dma_start(
                        out=yt[:, :, c * TCH:(c + 1) * TCH]
                        .rearrange("e p t -> p e t"),
                        in_=y_stage[c][:])

            def emit_proj_tail(c):
                # et-pairs with deferred sp3: PE accumulates stacks 0-2 of
                # both ets while the last hp's normalize (which produces
                # stack 3) drains on DVE; copies alternate DVE/ACT; y goes
                # out in two half-chunk DMAs.
                ensure_ystage(c)
                for pair in range(ND // 2):
                    e0, e1 = 2 * pair, 2 * pair + 1
                    pss = {e0: psW.tile([P, TCH], F32, tag="w", name="y_ps"),
                           e1: psO.tile([P, TCH], F32, tag="o", name="o_ps")}
                    for et in (e0, e1):
                        for sp in range(NSP - 1):
                            nc.tensor.matmul(
                                pss[et][:],
                                wot_sb[:, sp, et * P:(et + 1) * P],
                                o_tiles[c][:, sp, :],
                                start=(sp == 0), stop=False)
                    for et in (e0, e1):
                        nc.tensor.matmul(
                            pss[et][:],
                            wot_sb[:, NSP - 1, et * P:(et + 1) * P],
                            o_tiles[c][:, NSP - 1, :],
                            start=False, stop=True)
                        cp = nc.vector.tensor_copy if et % 2 == 0 else cp_act
                        cp(y_stage[c][:, et, :], pss[et][:])
                    if pair == 2:
                        nc.sync.dma_start(
                            out=yt[0:6, :, c * TCH:(c + 1) * TCH]
                            .rearrange("e p t -> p e t"),
                            in_=y_stage[c][:, 0:6, :])
                    elif pair == 3:
                        nc.sync.dma_start(
                            out=yt[6:8, :, c * TCH:(c + 1) * TCH]
                            .rearrange("e p t -> p e t"),
                            in_=y_stage[c][:, 6:8, :])

            def emit_hp(c, hp, o_sb_c, pull):
                qt_c = qt_tiles[c]
                nkt = 4 * c + 4
                o_ps = [psO.tile([P, TCH], F32, tag="o", name="o_ps")
                        for _ in range(2)]

                def emit_av(kt, pt, q0):
                    cc, tt = kt // 4, kt % 4
                    for j in range(2):
                        h = 2 * hp + j
                        nc.tensor.matmul(
                            o_ps[j][0:HS + 1, q0:],
                            v_sb[cc][:, tt, h, :],
                            pt[:, j, q0:],
                            start=(kt == 0), stop=(kt == nkt - 1))

                # software-pipelined: scores(kt+1)/(kt+2) enter the PE queue
                # BEFORE av(kt) so the exp(kt) + mask(kt) latency hides
                # behind them; one filler quantum is woven in per kt step.
                pends = []
                for kt in range(nkt):
                    cc, tt = kt // 4, kt % 4
                    q0 = max(0, P * kt - TCH * c)
                    s_ps = psS.tile([P, 2, TCH], F32, tag="s", name="s_ps")
                    for j in range(2):
                        h = 2 * hp + j
                        hb = (h % 4) * 32
                        g = h // 4
                        nc.tensor.matmul(
                            s_ps[:, j, q0:],
                            kt_sb[cc][hb:hb + 32, g, :, tt * P:(tt + 1) * P],
                            qt_c[hb:hb + 32, g, :, q0:],
                            start=True, stop=True, perf_mode=DROW,
                            tile_position=(hb, 0))
                    pt = ptp.tile([P, 2, TCH], mmdt, tag="pt", name="pt")
                    nc.scalar.activation(
                        pt[:, :, q0:], s_ps[:, :, q0:], EXP, scale=float(SCALE))
                    if kt >= 4 * c:  # diagonal block: causal tri mask
                        # on GpSimd/Pool: SBUF-only op, keeps the in-order
                        # DVE queue free of latency-critical work
                        for j in range(2):
                            nc.gpsimd.tensor_mul(
                                pt[:, j, q0:q0 + P], pt[:, j, q0:q0 + P],
                                tri_sb[:])
                    if len(pends) >= 2:
                        emit_av(*pends.pop(0))
                    pends.append((kt, pt, q0))
                    pull()
                for pend in pends:
                    emit_av(*pend)
                # normalize: rows 0:64 of o_ps / row 64 (ones-column rowsum).
                # All-DVE chain: recip -> 2x stream_shuffle lane-0 broadcast
                # -> mul.  rcp/bc in bf16 so the shuffles hit the fast
                # 16-bit DVE modes.
                for j in range(2):
                    nc.vector.reciprocal(rcp_st[64:65, :], o_ps[j][64:65, :])
                    bc_sb = bcp.tile([P, TCH], F32, tag="bc", name="bc_sb")
                    nc.vector.stream_shuffle(
                        bc_sb[0:32, :], rcp_st[64:96, :], [0] * 32)
                    nc.vector.stream_shuffle(
                        bc_sb[32:64, :], rcp_st[64:96, :], [0] * 32)
                    nc.vector.tensor_mul(
                        o_sb_c[j * 64:(j + 1) * 64, hp, :],
                        o_ps[j][0:64, :], bc_sb[0:64, :])

            def emit_body():
                # Startup: only what chunk-0's first two head-pairs (group
                # g0) need — Q0/K0 g0 stacks + V0; the g1 stacks weave into
                # chunk 0 (deadline: hp2).  First Q pair dt-outer for the
                # earliest possible PE start; copies alternate DVE/ACT.
                emit_x_dma(0)
                emit_qk0_pair(0, 0)
                cps = [nc.vector.tensor_copy, cp_act]
                pools = [psO, psO, psW]
                for i, (which, st) in enumerate([(1, 0), (1, 1)]):
                    emit_qk_stack(0, which, st, cp=cps[i % 2],
                                  pool=pools[i % 3])
                for tt in range(NTT):
                    emit_v_tt(0, tt, pool=pools[tt % 3])

                # Filler schedule.  Attention is ACT(exp)-paced after the
                # fp8 scores change (~0.5us/kt PE deficit), so fillers are
                # paced at that rate with explicit deadlines:
                #  - K(c)/V(c) weave inside chunk c (needed by its diag kts)
                #  - Q(c+1) weaves in chunk c (needed at chunk c+1 start)
                #  - proj(c) defers into the merged phase
                # Chunks 2+3 run as ONE interleaved phase (hp-alternating)
                # so chunk 3's huge exp load shares chunk 2's filler pool.
                dve_cp = nc.vector.tensor_copy
                PC = 0.426  # fine filler piece cost (us): 2 matmuls

                def qk(c, which, st, dl, cp=None):
                    # four quarter-stack pieces sharing one PSUM accumulation
                    cp = cp or dve_cp
                    return [(lambda p=p: emit_qk_stack(c, which, st, cp,
                                                       part=p), PC, dl)
                            for p in range(4)]

                def vq(c, tt, dl):
                    return [(lambda p=p: emit_v_tt(c, tt, part=p), PC, dl)
                            for p in range(4)]

                def pj(c, et):
                    return [(lambda p=p: emit_proj_et(c, et, part=p), PC,
                             None) for p in range(2)]

                def flat(groups):
                    return [q for grp in groups for q in grp]

                RATE = 0.5
                # --- chunk 0 (16 calls) --- DVE is loaded with normalize
                # here while ACT has slack: QK copies go to ACT.  V0 tt1-3
                # weave in with tight deadlines (diag kts 1-3 of hp0).
                o_tiles[0] = osbp.tile([P, NSP, TCH], mmdt, tag="o", name="o0")
                emit_x_dma(1)
                w = _Weaver(flat([qk(0, 0, 2, 9, cp_act), qk(0, 1, 2, 9, cp_act),
                                  qk(0, 0, 3, 9, cp_act), qk(0, 1, 3, 9, cp_act),
                                  qk(1, 0, 0, 14, cp_act), qk(1, 0, 1, 14, cp_act),
                                  qk(1, 0, 2, None, cp_act),
                                  qk(1, 0, 3, None, cp_act)]),
                            RATE)
                for hp in range(4):
                    emit_hp(0, hp, o_tiles[0], w)
                w.drain()
                # --- chunk 1 (32 calls) ---
                o_tiles[1] = osbp.tile([P, NSP, TCH], mmdt, tag="o", name="o1")
                emit_x_dma(2)
                emit_x_dma(3)
                w = _Weaver(flat([qk(1, 1, 0, 5, cp_act), qk(1, 1, 1, 5, cp_act),
                                  vq(1, 0, 5), vq(1, 1, 6), vq(1, 2, 7),
                                  vq(1, 3, 8),
                                  qk(1, 1, 2, 21, cp_act),
                                  qk(1, 1, 3, 21, cp_act),
                                  qk(2, 0, 0, 30, cp_act),
                                  qk(2, 0, 1, 30, cp_act),
                                  qk(2, 0, 2, None, cp_act),
                                  qk(2, 0, 3, None, cp_act)]),
                            RATE)
                for hp in range(4):
                    emit_hp(1, hp, o_tiles[1], w)
                w.drain()
                # --- merged chunks 2+3 (112 calls, hp-alternating) ---
                o_tiles[2] = osbp.tile([P, NSP, TCH], mmdt, tag="o", name="o2")
                o_tiles[3] = osbp.tile([P, NSP, TCH], mmdt, tag="o", name="o3")
                quanta = flat([qk(2, 1, 0, 9), qk(2, 1, 1, 9),
                               vq(2, 0, 9), vq(2, 1, 10), vq(2, 2, 11),
                               vq(2, 3, 12),
                               qk(3, 0, 0, 13), qk(3, 0, 1, 13),
                               qk(3, 1, 0, 25), qk(3, 1, 1, 25),
                               vq(3, 0, 25), vq(3, 1, 26), vq(3, 2, 27),
                               vq(3, 3, 28),
                               qk(2, 1, 2, 37), qk(2, 1, 3, 37),
                               qk(3, 0, 2, 41), qk(3, 0, 3, 41),
                               qk(3, 1, 2, 53), qk(3, 1, 3, 53)]
                              + [pj(0, et) for et in range(ND)]
                              + [pj(1, et) for et in range(ND)]
                              + [pj(2, et) for et in range(ND)])
                w = _Weaver(quanta, 0.58)
                for hp in range(4):
                    emit_hp(2, hp, o_tiles[2], w)
                    emit_hp(3, hp, o_tiles[3], w)
                w.drain()

                # tail: last chunk's projection. sp3 (the stack normalized
                # last) is deferred per et-pair so PE keeps busy on the
                # other stacks while the final normalize chain drains.
                emit_proj_tail(NCHUNK - 1)

            import contextlib
            loop_ctx = (tc.For_i(0, reps, 1) if reps > 1
                        else contextlib.nullcontext())
            with loop_ctx:
                emit_body()

    nc.compile()
    return nc


_CACHE = {}


def _get_program(reps: int = 1, mmdt=BF16):
    key = ("nc", reps, str(mmdt))
    if key not in _CACHE:
        _CACHE[key] = build_program(reps, mmdt)
    return _CACHE[key]


def make_in_maps(x, Wq, Wk, Wv, Wo, npdt=None):
    if npdt is None:
        import ml_dtypes
        npdt = ml_dtypes.bfloat16
    x = np.asarray(x, dtype=np.float32)
    Wq = np.asarray(Wq, dtype=np.float32)
    Wk = np.asarray(Wk, dtype=np.float32)
    Wv = np.asarray(Wv, dtype=np.float32)
    Wo = np.asarray(Wo, dtype=np.float32)
    tri = np.triu(np.ones((P, P), dtype=np.float32))

    def wmat(W, g):
        # [H_LOC, D, HS] -> [D, S] (s = h_local*HS + e) -> p-major [P, ND, S]
        m = W[g * H_LOC:(g + 1) * H_LOC].transpose(1, 0, 2).reshape(D, S)
        return np.ascontiguousarray(m.reshape(ND, P, S).transpose(1, 0, 2))

    # fp8-DoubleRow packing permutation for wq/wk: stack st = (grp, subrow),
    # column c -> head 4*grp + c//32, head-dim element e = 2*(c%32) + subrow
    perm = np.empty(S, dtype=np.int64)
    for st in range(4):
        grp, r = st // 2, st % 2
        for cc in range(P):
            perm[st * P + cc] = (4 * grp + cc // 32) * HS + 2 * (cc % 32) + r

    def wmat_qk(W, g):
        m = W[g * H_LOC:(g + 1) * H_LOC].transpose(1, 0, 2).reshape(D, S)
        m = m[:, perm]
        return np.ascontiguousarray(m.reshape(ND, P, S).transpose(1, 0, 2))

    in_maps = []
    for core in range(NCORES):
        b, g = core // HG, core % HG
        xT = x[b].T  # [D, T]
        xt_t = np.ascontiguousarray(
            xT.reshape(ND, P, NCHUNK, TCH).transpose(2, 1, 0, 3))
        woT = Wo[:, g * S:(g + 1) * S].T  # [S, D]
        wot_t = np.ascontiguousarray(woT.reshape(NSP, P, D).transpose(1, 0, 2))
        in_maps.append({
            "xt": xt_t.astype(npdt),
            "wq": wmat_qk(Wq, g).astype(npdt),
            "wk": wmat_qk(Wk, g).astype(npdt),
            "wv": wmat(Wv, g).astype(npdt),
            "wot": wot_t.astype(npdt),
            "tri": tri.astype(npdt),
        })
    return in_maps


def kernel_ex(x, Wq, Wk, Wv, Wo, bo, **run_kwargs):
    """Run and return (output, BassKernelResults)."""
    nc = _get_program()
    in_maps = make_in_maps(x, Wq, Wk, Wv, Wo)
    res = run_bass_kernel_spmd(nc, in_maps, core_ids=list(range(NCORES)),
                               **run_kwargs)
    bo = np.asarray(bo, dtype=np.float32)
    y = np.empty((B, T, D), dtype=np.float32)
    for b in range(B):
        y0 = res.results[HG * b]["yt"].astype(np.float32).reshape(D, T)
        y1 = res.results[HG * b + 1]["yt"].astype(np.float32).reshape(D, T)
        y[b] = (y0 + y1).T + bo
    return y, res


def kernel(x, Wq, Wk, Wv, Wo, bo):
    y, _ = kernel_ex(x, Wq, Wk, Wv, Wo, bo)
    return y


# revision 90
# speedup vs baseline: 1.0186x; 1.0086x over previous
"""Trainium2 Bass kernel for nn_MultiHeadAttention_66984309948505.

Full causal MHA: x[4,2048,1024], 16 heads of 64, out-proj + bias.

Sharding (8 cores): 4-way data-parallel over batch x 2-way tensor-parallel
over heads. Core (b, g) computes heads [8g, 8g+8) for batch b, including the
partial output projection Y_partial = O_g @ Wo[:, 512g:512(g+1)].T.
Host-side unshard: Y[b] = (Y_partial[b,g=0] + Y_partial[b,g=1]).T + bo.

Device layouts are all "transposed" (feature-major) so no on-chip transposes
are needed anywhere:
  xt  [c, p, dt, s] p-major tiling of x[b].T    (host pre-transpose)
  QT/KT fp8e4m3, DoubleRow-packed [128p = 4 heads x 32 lanes, grp, sub, T]:
                 head-dim element e of head h lives at partition
                 (h%4)*32 + e//2, group h//4, subrow e%2.  The host permutes
                 wq/wk s-columns so the projection matmuls PRODUCE this
                 layout directly (no repack).  Scores then run as fp8
                 DoubleRow matmuls (0.5 PE cycles/row — 2x bf16 rate).
  scores S^T[k, q] per head-pair; softmax denominator comes free from a
                 ones-column appended to V in the AV matmul (PSUM row 64)
  O^T [S, T], Y^T [D, T] in bf16 -> host transposes back.

Everything else is bf16 operands with fp32 PSUM accumulation (rel err
~9.8e-3, dominated by the fp8 Q/K quantization; gate is 2e-2).

Scheduling: attention is Activation(exp)-paced, so QKV-projection and
output-projection work is chopped into ~426ns "filler pieces" woven between
attention kt-steps by a rate+deadline Weaver; chunks 2+3 run hp-interleaved
as one merged phase so chunk 3's exp load shares chunk 2's filler pool.
Engine map: PE matmuls; ACT exp + some copies; DVE normalize chain + most
PSUM->SBUF copies; GpSimd/Pool causal-mask muls + memsets (Pool cannot
access PSUM on real HW).  DMAs are batched (each holds the single HWDGE
dispatcher ~625ns); y stores stage through SBUF and go out per chunk.
"""

import numpy as np

import concourse.bacc as bacc
import concourse.bass as bass
import concourse.mybir as mybir
import concourse.tile as tile
from concourse.bass_utils import run_bass_kernel_spmd

# Problem constants (hardcoded per contract)
B, T, D = 4, 2048, 1024
H, HS = 16, 64
NCORES = 8
HG = 2                 # head-group TP degree
H_LOC = H // HG        # 8 heads per core
S = H_LOC * HS         # 512 local head dims
P = 128
TCH = 512              # t/q chunk width
NCHUNK = T // TCH      # 4
ND = D // P            # 8 d-tiles
NSP = S // P           # 4 head stacks
NTT = TCH // P         # 4 k-subtiles per chunk
SCALE = 1.0 / np.sqrt(HS)

F32 = mybir.dt.float32
BF16 = mybir.dt.bfloat16
F8 = mybir.dt.float8e4
EXP = mybir.ActivationFunctionType.Exp
CPY = mybir.ActivationFunctionType.Copy
DROW = mybir.MatmulPerfMode.DoubleRow
NG = H_LOC // 4        # 2 four-head groups per core


class _Weaver:
    """Rate- and deadline-paced filler emission.  Each quantum is
    (closure, cost_us, deadline_call|None).  At pull() number k, emits (in
    list order) everything whose deadline is within `lead` calls, then keeps
    emitting while the cumulative cost is below rate_us * k."""

    def __init__(self, quanta, rate_us, lead=3):
        self.q = list(quanta)
        self.rate = rate_us
        self.lead = lead
        self.emitted = 0
        self.cost = 0.0
        self.calls = 0

    def _emit_next(self):
        fn, cost, _ = self.q[self.emitted]
        fn()
        self.cost += cost
        self.emitted += 1

    def __call__(self):
        self.calls += 1
        while self.emitted < len(self.q):
            _, cost, dl = self.q[self.emitted]
            due = dl is not None and dl <= self.calls + self.lead
            if due or self.cost < self.rate * self.calls:
                self._emit_next()
            else:
                break

    def drain(self):
        while self.emitted < len(self.q):
            self._emit_next()


def build_program(reps: int = 1, mmdt=BF16):
    nc = bacc.Bacc("TRN2", target_bir_lowering=False, debug=False)

    xt = nc.dram_tensor("xt", [NCHUNK, P, ND, TCH], mmdt, kind="ExternalInput")
    wq = nc.dram_tensor("wq", [P, ND, S], mmdt, kind="ExternalInput")
    wk = nc.dram_tensor("wk", [P, ND, S], mmdt, kind="ExternalInput")
    wv = nc.dram_tensor("wv", [P, ND, S], mmdt, kind="ExternalInput")
    wot = nc.dram_tensor("wot", [P, NSP, D], mmdt, kind="ExternalInput")
    tri = nc.dram_tensor("tri", [P, P], mmdt, kind="ExternalInput")
    yt = nc.dram_tensor("yt", [ND, P, T], mmdt, kind="ExternalOutput")

    with tile.TileContext(nc) as tc:
        with (
            nc.allow_low_precision(reason="bf16 matmul operands, fp32 accum"),
            tc.tile_pool(name="const", bufs=1) as constp,
            tc.tile_pool(name="kv", bufs=1) as kvp,
            tc.tile_pool(name="qt", bufs=2) as qtp,
            tc.tile_pool(name="osb", bufs=4) as osbp,
            tc.tile_pool(name="xp", bufs=3) as xp,
            tc.tile_pool(name="ptp", bufs=3) as ptp,
            tc.tile_pool(name="bcp", bufs=2) as bcp,
            tc.tile_pool(name="stg", bufs=2) as stp,
            tc.tile_pool(name="psS", bufs=2, space="PSUM") as psS,
            tc.tile_pool(name="psO", bufs=3, space="PSUM") as psO,
            tc.tile_pool(name="psW", bufs=1, space="PSUM") as psW,
        ):
            # Resident weights / constants
            tri_sb = constp.tile([P, P], mmdt, name="tri_sb")
            rcp_st = constp.tile([P, TCH], F32, name="rcp_st")
            nc.gpsimd.memset(rcp_st[:], 1.0)
            # PE p-state warmup: junk matmuls on memset data burn the
            # 0.65/1.2GHz ramp during the initial DMA wait, so the first
            # real matmuls run at full clock
            warm_sb = constp.tile([P, 256], mmdt, name="warm_sb")
            nc.gpsimd.memset(warm_sb[:], 0.0)
            warm_ps = psW.tile([P, TCH], F32, tag="w", name="warm_ps")
            for i in range(8):
                nc.tensor.matmul(warm_ps[:, 0:256], warm_sb[:, 0:P], warm_sb[:],
                                 start=(i == 0), stop=(i == 8 - 1))
            wq_sb = constp.tile([P, ND, S], mmdt, name="wq_sb")
            wk_sb = constp.tile([P, ND, S], mmdt, name="wk_sb")
            wv_sb = constp.tile([P, ND, S], mmdt, name="wv_sb")
            wot_sb = constp.tile([P, NSP, D], mmdt, name="wot_sb")

            xt_tiles = {}

            def emit_x_dma(c):
                if c not in xt_tiles:
                    xt_tiles[c] = xp.tile([P, ND, TCH], mmdt, tag="x",
                                          name=f"x{c}")
                    nc.sync.dma_start(out=xt_tiles[c][:], in_=xt[c])

            if reps == 1:
                # Startup: interleave x0 / wq in small pieces so the
                # dt-outer chunk-0 Q loop starts ~3us in; the g0 half of wk
                # comes early (K0 g0-stacks run right after Q0 g0), then wv
                # (V0 weaves into chunk 0 with tight deadlines), rest after.
                xt_tiles[0] = xp.tile([P, ND, TCH], mmdt, tag="x", name="x0")
                nc.sync.dma_start(out=xt_tiles[0][:, 0:1, 0:256],
                                  in_=xt[0, :, 0:1, 0:256])
                nc.sync.dma_start(out=wq_sb[:, 0:1, 0:2 * P],
                                  in_=wq[:, 0:1, 0:2 * P])
                nc.sync.dma_start(out=xt_tiles[0][:, 0:1, 256:TCH],
                                  in_=xt[0, :, 0:1, 256:TCH])
                sl = slice(1, 4)
                nc.sync.dma_start(out=xt_tiles[0][:, sl, :],
                                  in_=xt[0, :, sl, :])
                nc.sync.dma_start(out=wq_sb[:, sl, 0:2 * P],
                                  in_=wq[:, sl, 0:2 * P])
                nc.sync.dma_start(out=wk_sb[:, :, 0:2 * P],
                                  in_=wk[:, :, 0:2 * P])
                sl = slice(4, 8)
                nc.sync.dma_start(out=xt_tiles[0][:, sl, :],
                                  in_=xt[0, :, sl, :])
                nc.sync.dma_start(out=wq_sb[:, sl, 0:2 * P],
                                  in_=wq[:, sl, 0:2 * P])
                nc.sync.dma_start(out=wv_sb[:, 0:2, :], in_=wv[:, 0:2, :])
                nc.sync.dma_start(out=wv_sb[:, 2:4, :], in_=wv[:, 2:4, :])
                nc.sync.dma_start(out=wv_sb[:, 4:8, :], in_=wv[:, 4:8, :])
                nc.sync.dma_start(out=wq_sb[:, :, 2 * P:],
                                  in_=wq[:, :, 2 * P:])
                nc.sync.dma_start(out=wk_sb[:, :, 2 * P:],
                                  in_=wk[:, :, 2 * P:])
            else:
                nc.sync.dma_start(out=wq_sb[:], in_=wq[:])
                nc.sync.dma_start(out=wk_sb[:], in_=wk[:])
                nc.sync.dma_start(out=wv_sb[:, 0:2, :], in_=wv[:, 0:2, :])
                nc.sync.dma_start(out=wv_sb[:, 2:4, :], in_=wv[:, 2:4, :])
                nc.sync.dma_start(out=wv_sb[:, 4:8, :], in_=wv[:, 4:8, :])
            nc.sync.dma_start(out=tri_sb[:], in_=tri[:])
            nc.sync.dma_start(out=wot_sb[:], in_=wot[:])

            # Resident K^T and V (per-chunk tiles for clean dep tracking);
            # V column 64 = 1.0 -> AV PSUM row 64 is the softmax denominator.
            # Q^T/K^T live in fp8e4m3 packed for DoubleRow score matmuls:
            # [128p = 4 heads x 32 lanes, group, subrow, T], head-dim element
            # e of head h at (partition (h%4)*32 + e//2, group h//4, sub e%2).
            kt_sb = [kvp.tile([P, NG, 2, TCH], F8, name=f"kt{c}")
                     for c in range(NCHUNK)]
            v_sb = [kvp.tile([P, NTT, H_LOC, HS + 1], mmdt, name=f"v{c}")
                    for c in range(NCHUNK)]
            for c in range(NCHUNK):
                nc.gpsimd.memset(v_sb[c][:, :, :, HS:HS + 1], 1.0)

            qt_tiles = {}
            o_tiles = {}
            y_stage = {}

            def ensure_qt(c):
                if c not in qt_tiles:
                    qt_tiles[c] = qtp.tile([P, NG, 2, TCH], F8, tag="qt",
                                           name=f"qt{c}")

            def cp_act(dst, src):
                nc.scalar.activation(dst, src, CPY)

            pending_ps = {}

            def emit_qk_stack(c, which, st, cp=cp_act, pool=None, part=None,
                              nparts=4):
                # stack st = (group g = st//2, subrow r = st%2); the host
                # permutes wq/wk s-columns to match, so the matmul output
                # rows land directly in fp8-DoubleRow packing order.
                # part=i of nparts emits a dt-slice of the shared PSUM
                # accumulation so filler weaves at fine (~426ns) grain.
                ensure_qt(c)
                w_sb, dst = ((wq_sb, qt_tiles[c]), (wk_sb, kt_sb[c]))[which]
                pool = pool or psW
                key = ("qk", c, which, st)
                if part in (None, 0):
                    ps = (pool.tile([P, TCH], F32, tag="w", name="ps_a")
                          if pool is psW else
                          pool.tile([P, TCH], F32, tag="o", name="o_ps"))
                else:
                    ps = pending_ps.pop(key)
                step = ND // nparts if part is not None else ND
                d0 = 0 if part is None else part * step
                for dt in range(d0, d0 + step):
                    nc.tensor.matmul(
                        ps[:], w_sb[:, dt, st * P:(st + 1) * P],
                        xt_tiles[c][:, dt, :],
                        start=(dt == 0), stop=(dt == ND - 1))
                if part is not None and part < nparts - 1:
                    pending_ps[key] = ps
                else:
                    cp(dst[:, st // 2, st % 2, :], ps[:])

            def emit_qk0_pair(which, pair):
                # dt-outer over two stacks: round dt only needs the dt-piece
                # of (x0, w) -> PE starts as soon as the first DMAs land.
                # Borrows a psO slot for the second live accumulator.
                ensure_qt(0)
                w_sb, dst = ((wq_sb, qt_tiles[0]), (wk_sb, kt_sb[0]))[which]
                pss = [psW.tile([P, TCH], F32, tag="w", name="ps_a"),
                       psO.tile([P, TCH], F32, tag="o", name="o_ps")]
                for i in range(2):
                    st = 2 * pair + i
                    for cols in (slice(0, 256), slice(256, TCH)):
                        nc.tensor.matmul(
                            pss[i][:, cols], w_sb[:, 0, st * P:(st + 1) * P],
                            xt_tiles[0][:, 0, cols],
                            start=True, stop=False)
                for dt in range(1, ND):
                    for i in range(2):
                        st = 2 * pair + i
                        nc.tensor.matmul(
                            pss[i][:], w_sb[:, dt, st * P:(st + 1) * P],
                            xt_tiles[0][:, dt, :],
                            start=False, stop=(dt == ND - 1))
                for i in range(2):
                    st = 2 * pair + i
                    (cp_act if i else nc.vector.tensor_copy)(
                        dst[:, st // 2, st % 2, :], pss[i][:])

            def emit_v_tt(c, tt, pool=None, part=None, nparts=4):
                pool = pool or psW
                key = ("v", c, tt)
                if part in (None, 0):
                    ps = (pool.tile([P, TCH], F32, tag="w", name="ps_v")
                          if pool is psW else
                          pool.tile([P, TCH], F32, tag="o", name="o_ps"))
                else:
                    ps = pending_ps.pop(key)
                step = ND // nparts if part is not None else ND
                d0 = 0 if part is None else part * step
                for dt in range(d0, d0 + step):
                    nc.tensor.matmul(
                        ps[:], xt_tiles[c][:, dt, tt * P:(tt + 1) * P],
                        wv_sb[:, dt, :],
                        start=(dt == 0), stop=(dt == ND - 1))
                if part is not None and part < nparts - 1:
                    pending_ps[key] = ps
                else:
                    nc.vector.tensor_copy(
                        v_sb[c][:, tt, :, 0:HS],
                        ps[:].rearrange("p (h e) -> p h e", h=H_LOC))

            def ensure_ystage(c):
                if c not in y_stage:
                    y_stage[c] = stp.tile([P, ND, TCH], mmdt, tag="y",
                                          name=f"yst{c}")

            def emit_proj_et(c, et, part=None):
                ensure_ystage(c)
                key = ("p", c, et)
                if part in (None, 0):
                    y_ps = psW.tile([P, TCH], F32, tag="w", name="y_ps")
                else:
                    y_ps = pending_ps.pop(key)
                sps = (range(NSP) if part is None else
                       range(0, 2) if part == 0 else range(2, NSP))
                for sp in sps:
                    nc.tensor.matmul(
                        y_ps[:], wot_sb[:, sp, et * P:(et + 1) * P],
                        o_tiles[c][:, sp, :],
                        start=(sp == 0), stop=(sp == NSP - 1))
                if part == 0:
                    pending_ps[key] = y_ps
                    return
                nc.vector.tensor_copy(y_stage[c][:, et, :], y_ps[:])
                if et == ND - 1:
                    nc.sync.dma_start(
                        out=yt[:, :, c * TCH:(c + 1) * TCH]
                        .rearrange("e p t -> p e t"),
                        in_=y_stage[c][:])

            def emit_proj_tail(c):
                # et-pairs with deferred sp3: PE accumulates stacks 0-2 of
                # both ets while the last hp's normalize (which produces
                # stack 3) drains on DVE; copies alternate DVE/ACT; y goes
                # out in two half-chunk DMAs.
                ensure_ystage(c)
                for pair in range(ND // 2):
                    e0, e1 = 2 * pair, 2 * pair + 1
                    pss = {e0: psW.tile([P, TCH], F32, tag="w", name="y_ps"),
                           e1: psO.tile([P, TCH], F32, tag="o", name="o_ps")}
                    for et in (e0, e1):
                        for sp in range(NSP - 1):
                            nc.tensor.matmul(
                                pss[et][:],
                                wot_sb[:, sp, et * P:(et + 1) * P],
                                o_tiles[c][:, sp, :],
                                start=(sp == 0), stop=False)
                    for et in (e0, e1):
                        nc.tensor.matmul(
                            pss[et][:],
                            wot_sb[:, NSP - 1, et * P:(et + 1) * P],
                            o_tiles[c][:, NSP - 1, :],
                            start=False, stop=True)
                        cp = nc.vector.tensor_copy if et % 2 == 0 else cp_act
                        cp(y_stage[c][:, et, :], pss[et][:])
                    if pair == 2:
                        nc.sync.dma_start(
                            out=yt[0:6, :, c * TCH:(c + 1) * TCH]
                            .rearrange("e p t -> p e t"),
                            in_=y_stage[c][:, 0:6, :])
                    elif pair == 3:
                        nc.sync.dma_start(
                            out=yt[6:7, :, c * TCH:(c + 1) * TCH]
                            .rearrange("e p t -> p e t"),
                            in_=y_stage[c][:, 6:7, :])
                        nc.sync.dma_start(
                            out=yt[7:8, :, c * TCH:(c + 1) * TCH]
                            .rearrange("e p t -> p e t"),
                            in_=y_stage[c][:, 7:8, :])

            def emit_hp(c, hp, o_sb_c, pull):
                qt_c = qt_tiles[c]
                nkt = 4 * c + 4
                o_ps = [psO.tile([P, TCH], F32, tag="o", name="o_ps")
                        for _ in range(2)]

                def emit_av(kt, pt, q0):
                    cc, tt = kt // 4, kt % 4
                    for j in range(2):
                        h = 2 * hp + j
                        nc.tensor.matmul(
                            o_ps[j][0:HS + 1, q0:],
                            v_sb[cc][:, tt, h, :],
                            pt[:, j, q0:],
                            start=(kt == 0), stop=(kt == nkt - 1))

                # software-pipelined: scores(kt+1)/(kt+2) enter the PE queue
                # BEFORE av(kt) so the exp(kt) + mask(kt) latency hides
                # behind them; one filler quantum is woven in per kt step.
                pends = []
                for kt in range(nkt):
                    cc, tt = kt // 4, kt % 4
                    q0 = max(0, P * kt - TCH * c)
                    s_ps = psS.tile([P, 2, TCH], F32, tag="s", name="s_ps")
                    for j in range(2):
                        h = 2 * hp + j
                        hb = (h % 4) * 32
                        g = h // 4
                        nc.tensor.matmul(
                            s_ps[:, j, q0:],
                            kt_sb[cc][hb:hb + 32, g, :, tt * P:(tt + 1) * P],
                            qt_c[hb:hb + 32, g, :, q0:],
                            start=True, stop=True, perf_mode=DROW,
                            tile_position=(hb, 0))
                    pt = ptp.tile([P, 2, TCH], mmdt, tag="pt", name="pt")
                    nc.scalar.activation(
                        pt[:, :, q0:], s_ps[:, :, q0:], EXP, scale=float(SCALE))
                    if kt >= 4 * c:  # diagonal block: causal tri mask
                        # on GpSimd/Pool: SBUF-only op, keeps the in-order
                        # DVE queue free of latency-critical work
                        for j in range(2):
                            nc.gpsimd.tensor_mul(
                                pt[:, j, q0:q0 + P], pt[:, j, q0:q0 + P],
                                tri_sb[:])
                    if len(pends) >= 2:
                        emit_av(*pends.pop(0))
                    pends.append((kt, pt, q0))
                    pull()
                for pend in pends:
                    emit_av(*pend)
                # normalize: rows 0:64 of o_ps / row 64 (ones-column rowsum).
                # All-DVE chain: recip -> 2x stream_shuffle lane-0 broadcast
                # -> mul.  rcp/bc in bf16 so the shuffles hit the fast
                # 16-bit DVE modes.
                for j in range(2):
                    nc.vector.reciprocal(rcp_st[64:65, :], o_ps[j][64:65, :])
                    bc_sb = bcp.tile([P, TCH], F32, tag="bc", name="bc_sb")
                    nc.vector.stream_shuffle(
                        bc_sb[0:32, :], rcp_st[64:96, :], [0] * 32)
                    nc.vector.stream_shuffle(
                        bc_sb[32:64, :], rcp_st[64:96, :], [0] * 32)
                    nc.vector.tensor_mul(
                        o_sb_c[j * 64:(j + 1) * 64, hp, :],
                        o_ps[j][0:64, :], bc_sb[0:64, :])

            def emit_body():
                # Startup: only what chunk-0's first two head-pairs (group
                # g0) need — Q0/K0 g0 stacks + V0; the g1 stacks weave into
                # chunk 0 (deadline: hp2).  First Q pair dt-outer for the
                # earliest possible PE start; copies alternate DVE/ACT.
                emit_x_dma(0)
                emit_qk0_pair(0, 0)
                cps = [nc.vector.tensor_copy, cp_act]
                pools = [psO, psO, psW]
                for i, (which, st) in enumerate([(1, 0), (1, 1)]):
                    emit_qk_stack(0, which, st, cp=cps[i % 2],
                                  pool=pools[i % 3])
                for tt in range(NTT):
                    emit_v_tt(0, tt, pool=pools[tt % 3])

                # Filler schedule.  Attention is ACT(exp)-paced after the
                # fp8 scores change (~0.5us/kt PE deficit), so fillers are
                # paced at that rate with explicit deadlines:
                #  - K(c)/V(c) weave inside chunk c (needed by its diag kts)
                #  - Q(c+1) weaves in chunk c (needed at chunk c+1 start)
                #  - proj(c) defers into the merged phase
                # Chunks 2+3 run as ONE interleaved phase (hp-alternating)
                # so chunk 3's huge exp load shares chunk 2's filler pool.
                dve_cp = nc.vector.tensor_copy
                PC = 0.426  # fine filler piece cost (us): 2 matmuls

                def qk(c, which, st, dl, cp=None):
                    # four quarter-stack pieces sharing one PSUM accumulation
                    cp = cp or dve_cp
                    return [(lambda p=p: emit_qk_stack(c, which, st, cp,
                                                       part=p), PC, dl)
                            for p in range(4)]

                def vq(c, tt, dl):
                    return [(lambda p=p: emit_v_tt(c, tt, part=p), PC, dl)
                            for p in range(4)]

                def pj(c, et):
                    return [(lambda p=p: emit_proj_et(c, et, part=p), PC,
                             None) for p in range(2)]

                def flat(groups):
                    return [q for grp in groups for q in grp]

                RATE = 0.5
                # --- chunk 0 (16 calls) --- DVE is loaded with normalize
                # here while ACT has slack: QK copies go to ACT.  V0 tt1-3
                # weave in with tight deadlines (diag kts 1-3 of hp0).
                o_tiles[0] = osbp.tile([P, NSP, TCH], mmdt, tag="o", name="o0")
                emit_x_dma(1)
                w = _Weaver(flat([qk(0, 0, 2, 9, cp_act), qk(0, 1, 2, 9, cp_act),
                                  qk(0, 0, 3, 9, cp_act), qk(0, 1, 3, 9, cp_act),
                                  qk(1, 0, 0, 14, cp_act), qk(1, 0, 1, 14, cp_act),
                                  qk(1, 0, 2, None, cp_act),
                                  qk(1, 0, 3, None, cp_act)]),
                            RATE)
                for hp in range(4):
                    emit_hp(0, hp, o_tiles[0], w)
                w.drain()
                # --- chunk 1 (32 calls) ---
                o_tiles[1] = osbp.tile([P, NSP, TCH], mmdt, tag="o", name="o1")
                emit_x_dma(2)
                emit_x_dma(3)
                w = _Weaver(flat([qk(1, 1, 0, 5, cp_act), qk(1, 1, 1, 5, cp_act),
                                  vq(1, 0, 5), vq(1, 1, 6), vq(1, 2, 7),
                                  vq(1, 3, 8),
                                  qk(1, 1, 2, 21, cp_act),
                                  qk(1, 1, 3, 21, cp_act),
                                  qk(2, 0, 0, 30, cp_act),
                                  qk(2, 0, 1, 30, cp_act),
                                  qk(2, 0, 2, None, cp_act),
                                  qk(2, 0, 3, None, cp_act)]),
                            RATE)
                for hp in range(4):
                    emit_hp(1, hp, o_tiles[1], w)
                w.drain()
                # --- merged chunks 2+3 (112 calls, hp-alternating) ---
                o_tiles[2] = osbp.tile([P, NSP, TCH], mmdt, tag="o", name="o2")
                o_tiles[3] = osbp.tile([P, NSP, TCH], mmdt, tag="o", name="o3")
                quanta = flat([qk(2, 1, 0, 9), qk(2, 1, 1, 9),
                               vq(2, 0, 9), vq(2, 1, 10), vq(2, 2, 11),
                               vq(2, 3, 12),
                               qk(3, 0, 0, 13, cp_act), qk(3, 0, 1, 13, cp_act),
                               qk(3, 1, 0, 25), qk(3, 1, 1, 25),
                               vq(3, 0, 25), vq(3, 1, 26), vq(3, 2, 27),
                               vq(3, 3, 28),
                               qk(2, 1, 2, 37), qk(2, 1, 3, 37),
                               qk(3, 0, 2, 41), qk(3, 0, 3, 41),
                               qk(3, 1, 2, 53), qk(3, 1, 3, 53)]
                              + [pj(0, et) for et in range(ND)]
                              + [pj(1, et) for et in range(ND)]
                              + [pj(2, et) for et in range(ND)])
                w = _Weaver(quanta, 0.58)
                for hp in range(4):
                    emit_hp(2, hp, o_tiles[2], w)
                    emit_hp(3, hp, o_tiles[3], w)
                w.drain()

                # tail: last chunk's projection. sp3 (the stack normalized
                # last) is deferred per et-pair so PE keeps busy on the
                # other stacks while the final normalize chain drains.
                emit_proj_tail(NCHUNK - 1)

            import contextlib
            loop_ctx = (tc.For_i(0, reps, 1) if reps > 1
                        else contextlib.nullcontext())
            with loop_ctx:
                emit_body()

    nc.compile()
    return nc


_CACHE = {}


def _get_program(reps: int = 1, mmdt=BF16):
    key = ("nc", reps, str(mmdt))
    if key not in _CACHE:
        _CACHE[key] = build_program(reps, mmdt)
    return _CACHE[key]


def make_in_maps(x, Wq, Wk, Wv, Wo, npdt=None):
    if npdt is None:
        import ml_dtypes
        npdt = ml_dtypes.bfloat16
    x = np.asarray(x, dtype=np.float32)
    Wq = np.asarray(Wq, dtype=np.float32)
    Wk = np.asarray(Wk, dtype=np.float32)
    Wv = np.asarray(Wv, dtype=np.float32)
    Wo = np.asarray(Wo, dtype=np.float32)
    tri = np.triu(np.ones((P, P), dtype=np.float32))

    def wmat(W, g):
        # [H_LOC, D, HS] -> [D, S] (s = h_local*HS + e) -> p-major [P, ND, S]
        m = W[g * H_LOC:(g + 1) * H_LOC].transpose(1, 0, 2).reshape(D, S)
        return np.ascontiguousarray(m.reshape(ND, P, S).transpose(1, 0, 2))

    # fp8-DoubleRow packing permutation for wq/wk: stack st = (grp, subrow),
    # column c -> head 4*grp + c//32, head-dim element e = 2*(c%32) + subrow
    perm = np.empty(S, dtype=np.int64)
    for st in range(4):
        grp, r = st // 2, st % 2
        for cc in range(P):
            perm[st * P + cc] = (4 * grp + cc // 32) * HS + 2 * (cc % 32) + r

    def wmat_qk(W, g):
        m = W[g * H_LOC:(g + 1) * H_LOC].transpose(1, 0, 2).reshape(D, S)
        m = m[:, perm]
        return np.ascontiguousarray(m.reshape(ND, P, S).transpose(1, 0, 2))

    in_maps = []
    for core in range(NCORES):
        b, g = core // HG, core % HG
        xT = x[b].T  # [D, T]
        xt_t = np.ascontiguousarray(
            xT.reshape(ND, P, NCHUNK, TCH).transpose(2, 1, 0, 3))
        woT = Wo[:, g * S:(g + 1) * S].T  # [S, D]
        wot_t = np.ascontiguousarray(woT.reshape(NSP, P, D).transpose(1, 0, 2))
        in_maps.append({
            "xt": xt_t.astype(npdt),
            "wq": wmat_qk(Wq, g).astype(npdt),
            "wk": wmat_qk(Wk, g).astype(npdt),
            "wv": wmat(Wv, g).astype(npdt),
            "wot": wot_t.astype(npdt),
            "tri": tri.astype(npdt),
        })
    return in_maps


def kernel_ex(x, Wq, Wk, Wv, Wo, bo, **run_kwargs):
    """Run and return (output, BassKernelResults)."""
    nc = _get_program()
    in_maps = make_in_maps(x, Wq, Wk, Wv, Wo)
    res = run_bass_kernel_spmd(nc, in_maps, core_ids=list(range(NCORES)),
                               **run_kwargs)
    bo = np.asarray(bo, dtype=np.float32)
    y = np.empty((B, T, D), dtype=np.float32)
    for b in range(B):
        y0 = res.results[HG * b]["yt"].astype(np.float32).reshape(D, T)
        y1 = res.results[HG * b + 1]["yt"].astype(np.float32).reshape(D, T)
        y[b] = (y0 + y1).T + bo
    return y, res


def kernel(x, Wq, Wk, Wv, Wo, bo):
    y, _ = kernel_ex(x, Wq, Wk, Wv, Wo, bo)
    return y
